# revision 16
# baseline (speedup 1.0000x reference)
"""Trainium2 Bass kernel for CappedMean (segment_reduce).

Reference computation: out[b, d] = sum_{l < N[b]} x[b, l, d] / N[b]
with x: [2048, 512, 256] f32, N: [2048] int64 -> out: [2048, 256] f32.

Strategy:
  - Pure data parallel over the batch dim: 2048 / 8 cores = 256 batches/core.
  - Per batch, x[b] ([512, 256] f32 = 512 KB) is viewed as [128, 4, 256]:
    SBUF partition p holds rows l in {4p .. 4p+3}, so the HBM->SBUF DMA is
    perfectly linear (contiguous 4 KB per partition).
  - The masked reduction over l runs on the TensorEngine: for each sub-row
    j in 0..4, a [128,1]x[128,256] matmul with a 0/1 prefix-mask column as
    stationary weights accumulates into one PSUM row:
        psum[slot(b), d] += sum_p mask[b, 4p+j] * x[b, 4p+j, d]
    Masks are generated on-chip (iota + is_lt against broadcast N).
  - PSUM slot assignment works around PE write-port restrictions
    (M=1 outputs only at partitions 0/32/64/96; fp32r only partition 0):
      * f32 mode:  slot = (partition-group g, bank k), 4x4 = 16 in flight
                   per psum tile. Exact fp32 matmul (4 cycles/row).
      * f32r mode: slot = bank k on partition 0, 4 in flight per tile.
                   Single-pass matmul (1 cycle/row); the moving operand is
                   rounded to ~tf32 precision by the PE.
  - Two persistent 4-bank PSUM tiles double-buffer accumulate vs evict.
    Eviction (DVE) multiplies by 1/N and lands in SBUF; a strided DMA
    scatters rows back to the output layout.
"""

import sys

if "/opt/trn_rl_repo" not in sys.path:
    sys.path.insert(0, "/opt/trn_rl_repo")

import numpy as np

B, L, D = 2048, 512, 256
NCORES = 8
BSH = B // NCORES  # 256 batches per core
P = 128
J = L // P  # 4 sub-rows per partition
BT = BSH // P  # batch tiles per core
NG = 4  # partition groups in f32 mode (psum rows 0/32/64/96)
NK = 4  # psum banks per tile
BANK_F32 = 512  # one 2KB psum bank holds 512 f32

MM_MODE = "f32"  # "f32" exact 4cyc/row | "f32r" ~tf32 2cyc/row, psum part 0
#                  | "f16" cast-in-DMA, 1cyc/row, ~tf32-precision
X_BUFS = 16  # in-flight x tiles (BPD batches each)
BPD = 2  # batches per x DMA (1 MB transfers at 2)
ALT_DMA_ENGINES = False  # alternate sync/scalar HWDGE rings for the x stream


def build_program(n_bt: int = BT, mode: str = MM_MODE):
    import concourse.bacc as bacc
    import concourse.tile as tile
    from concourse import mybir
    from concourse.alu_op_type import AluOpType

    f32 = mybir.dt.float32
    mm_dt = {
        "f32": f32,
        "f32r": mybir.dt.float32r,
        "f16": mybir.dt.float16,
    }[mode]
    x_dram_dt = mm_dt if mode == "f32r" else f32
    bsh = n_bt * P

    nc = bacc.Bacc("TRN2", target_bir_lowering=False)
    x_d = nc.dram_tensor("x", [bsh, P, J * D], x_dram_dt, kind="ExternalInput")
    n_d = nc.dram_tensor("n", [n_bt, P], f32, kind="ExternalInput")
    if mode in ("f32", "f16"):
        r_d = nc.dram_tensor("r", [n_bt, P, P // (NG * NK), NK], f32,
                             kind="ExternalInput")
        r_ap = r_d[:]
    y_d = nc.dram_tensor("y", [bsh, D], f32, kind="ExternalOutput")
    x_ap, n_ap, y_ap = x_d[:], n_d[:], y_d[:]

    with tile.TileContext(nc) as tc:
        with (
            tc.tile_pool(name="const", bufs=1) as cpool,
            tc.tile_pool(name="small", bufs=2) as spool,
            tc.tile_pool(name="xin", bufs=X_BUFS) as xpool,
            tc.tile_pool(name="outp", bufs=2) as opool,
            tc.tile_pool(name="psum", bufs=1, space="PSUM") as ppool,
        ):
            # iota_f[p, j] = 4p + j = l  (row index within a batch)
            iota_i = cpool.tile([P, J], mybir.dt.int32)
            nc.gpsimd.iota(iota_i[:], pattern=[[1, J]], base=0, channel_multiplier=J)
            iota_f = cpool.tile([P, J], f32)
            nc.vector.tensor_copy(iota_f[:], iota_i[:])

            psum_ts = [
                ppool.tile([P, NK, BANK_F32], f32, name=f"ps{i}", tag=f"ps{i}")
                for i in range(2)
            ]
            if mode in ("f32", "f16"):
                # full-width eviction reads partitions the PE never writes
                for ps in psum_ts:
                    nc.vector.memset(ps[:], 0.0)

            for t in range(n_bt):
                n_row = spool.tile([1, P], f32)
                nc.sync.dma_start(out=n_row[:], in_=n_ap[t].unsqueeze(0))
                n_bc = spool.tile([P, P], f32)  # n_bc[p, b] = N[b]
                nc.gpsimd.partition_broadcast(n_bc[:], n_row[:])

                # mask[p, b, j] = 1.0 if (4p + j) < N[b] else 0.0
                mask = spool.tile([P, P, J], mm_dt)
                nc.vector.tensor_tensor(
                    mask[:],
                    iota_f[:].unsqueeze(1).broadcast_to([P, P, J]),
                    n_bc[:].unsqueeze(2).broadcast_to([P, P, J]),
                    AluOpType.is_lt,
                )

                if mode in ("f32", "f16"):
                    _emit_btile_gk(
                        nc, tc, t, x_ap, r_ap, y_ap, mask, psum_ts,
                        spool, xpool, opool, f32, mm_dt, AluOpType,
                    )
                else:
                    _emit_btile_f32r(
                        nc, tc, t, x_ap, n_row, y_ap, mask, psum_ts,
                        spool, xpool, opool, f32, AluOpType,
                    )

    nc.compile()
    return nc


def _emit_btile_gk(nc, tc, t, x_ap, r_ap, y_ap, mask, psum_ts,
                   spool, xpool, opool, f32, mm_dt, AluOpType):
    """16 batches in flight: slot (g, k) -> psum row 32g of bank k."""
    FG = NG * NK  # 16
    NF = P // FG  # 8 flight groups per batch tile
    cast = mm_dt != x_ap.dtype  # f16 mode: SWDGE casts f32 -> f16 in the DMA
    if cast:
        x_dmas = [nc.gpsimd]
    elif ALT_DMA_ENGINES:
        x_dmas = [nc.sync, nc.scalar]
    else:
        x_dmas = [nc.sync]
    # x viewed as [group, partition, batch-in-group, f] for BPD-batch DMAs
    xg_ap = x_ap.rearrange("(G u) p f -> G p u f", u=BPD)

    gpd = P // BPD  # x DMA groups per batch tile
    # Hoist the first flight's x DMAs ahead of the small n/rinv transfers so
    # the x stream starts as early as possible on the sync ring.
    xts_next = []
    for u in range(FG // BPD):
        grp = t * gpd + u
        xt = xpool.tile([P, BPD, J, D], mm_dt, name="xt", tag="xt")
        x_dmas[grp % len(x_dmas)].dma_start(out=xt[:], in_=xg_ap[grp])
        xts_next.append(xt)

    rinv = spool.tile([P, NF, NK], f32, name="rinv")
    nc.sync.dma_start(out=rinv[:], in_=r_ap[t])

    for F in range(NF):
        ps = psum_ts[(t * NF + F) % 2]
        xts = xts_next
        # prefetch next flight's x tiles
        xts_next = []
        if F + 1 < NF:
            for u in range(FG // BPD):
                grp = t * gpd + ((F + 1) * FG) // BPD + u
                xt = xpool.tile([P, BPD, J, D], mm_dt, name="xt", tag="xt")
                x_dmas[grp % len(x_dmas)].dma_start(out=xt[:], in_=xg_ap[grp])
                xts_next.append(xt)
        for i16 in range(FG):
            g, k = i16 // NK, i16 % NK
            bl = F * FG + i16
            xt = xts[i16 // BPD]
            for j in range(J):
                nc.tensor.matmul(
                    ps[32 * g : 32 * g + 1, k, 0:D],
                    mask[:, bl, j : j + 1],
                    xt[:, i16 % BPD, j, :],
                    start=(j == 0),
                    stop=(j == J - 1),
                    tile_position=(0, 32 * g),
                )
        # out_sb[:, k, d] = psum[:, k, d] * rinv  (only rows 32g are real)
        out_sb = opool.tile([P, NK, D], f32, name="out_sb", tag="out_sb")
        nc.vector.tensor_tensor(
            out_sb[:],
            ps[:, :, 0:D],
            rinv[:, F, :].unsqueeze(2).broadcast_to([P, NK, D]),
            AluOpType.mult,
        )
        # y rows bl = F*16 + g*4 + k  <-  out_sb[32g, k, :]
        src = out_sb[:].rearrange("(g r) k d -> g r k d", g=NG)[:, 0]
        dst = y_ap[t * P + F * FG : t * P + (F + 1) * FG, :].rearrange(
            "(g k) d -> g k d", g=NG
        )
        nc.sync.dma_start(out=dst, in_=src)


def _emit_btile_f32r(nc, tc, t, x_ap, n_row, y_ap, mask, psum_ts,
                     spool, xpool, opool, f32, AluOpType):
    """4 batches in flight per psum tile, all on psum partition 0."""
    NQ = 4  # output-staging groups per batch tile
    QB = P // NQ  # 32 batches per staging buffer
    FPQ = QB // NK  # 8 flights per staging buffer

    rinv_row = spool.tile([1, P], f32, name="rinv_row")
    nc.vector.reciprocal(rinv_row[:], n_row[:])

    for q in range(NQ):
        out_sb = opool.tile([1, QB, D], f32, name="out_sb_r", tag="out_sb_r")
        for fq in range(FPQ):
            F = q * FPQ + fq
            ps = psum_ts[(t * P // NK + F) % 2]
            for k in range(NK):
                bl = F * NK + k
                xt = xpool.tile([P, J, D], x_ap.dtype, name="xt", tag="xt")
                nc.sync.dma_start(out=xt[:], in_=x_ap[t * P + bl])
                for j in range(J):
                    nc.tensor.matmul(
                        ps[0:1, k, 0:D],
                        mask[:, bl, j : j + 1],
                        xt[:, j, :],
                        start=(j == 0),
                        stop=(j == J - 1),
                    )
            nc.vector.tensor_tensor(
                out_sb[0:1, fq * NK : (fq + 1) * NK, :],
                ps[0:1, :, 0:D],
                rinv_row[0:1, F * NK : (F + 1) * NK]
                .unsqueeze(2)
                .broadcast_to([1, NK, D]),
                AluOpType.mult,
            )
        nc.sync.dma_start(
            out=y_ap[t * P + q * QB : t * P + (q + 1) * QB, :].unsqueeze(0),
            in_=out_sb[:],
        )


def make_rinv(n_f32: np.ndarray) -> np.ndarray:
    """Host-side 1/N layout for f32-mode eviction: r[t, p, F, k] =
    1/N[t, F*16 + (p//32)*4 + k]."""
    n_bt = n_f32.shape[0]
    FG = NG * NK
    NF = P // FG
    r = np.empty((n_bt, P, NF, NK), dtype=np.float32)
    g = np.arange(P) // 32
    for t in range(n_bt):
        for F in range(NF):
            for k in range(NK):
                r[t, :, F, k] = 1.0 / n_f32[t, F * FG + g * NK + k]
    return r


_NC_CACHE = {}


def _get_nc():
    if "nc" not in _NC_CACHE:
        _NC_CACHE["nc"] = build_program()
    return _NC_CACHE["nc"]


def make_in_maps(x: np.ndarray, n: np.ndarray, mode: str = MM_MODE):
    xs = np.ascontiguousarray(x.astype(np.float32, copy=False)).reshape(
        NCORES, BSH, P, J * D
    )
    nf = np.asarray(n).astype(np.float32).reshape(NCORES, BT, P)
    maps = []
    for c in range(NCORES):
        m = {"x": xs[c], "n": nf[c]}
        if mode in ("f32", "f16"):
            m["r"] = make_rinv(nf[c])
        maps.append(m)
    return maps


def kernel(x, N):
    x = np.asarray(x)
    n = np.asarray(N)

    from concourse.bass_utils import run_bass_kernel_spmd

    nc = _get_nc()
    in_maps = make_in_maps(x, n)
    res = run_bass_kernel_spmd(nc, in_maps, core_ids=list(range(NCORES)))
    out = np.concatenate([r["y"] for r in res.results], axis=0)
    return out
